# revision 23
# baseline (speedup 1.0000x reference)
"""Causal self-attention (B=4, T=2048, C=1024, 16 heads) on 8 trn2 NeuronCores.

Sharding: tensor-parallel over heads (2 heads/core) for QKV + attention.
Output tokens are sharded *within each batch* (shard s owns tokens
[s*256, (s+1)*256) of every batch), so a per-batch AllToAll can fire as
soon as that batch's attention finishes and overlap with the next batch's
compute.  Each core returns out[4 batches x 256 tokens, C]; the host
reassembles.

Per-core pipeline (identical program on all cores; only the fed W_qkv
column-slice differs):
  stage 1: qT,kT [128ch x 2048tok] f32 and v [tok-major] bf16 per batch,
           from x @ W_qkv_slice (x DMA'd in natural layout, transposed on
           the PE in f32r).
  stage 2: causal attention per (batch, head): S^T tiles [kt=128, q=512]
           via f32r matmul (full-rate PE), exp on ACT -> bf16 probs,
           multiplicative 0/1 causal mask on diagonal blocks (DVE),
           AV accumulation in bf16 with a ones-column appended to v so
           PSUM row 64 carries the softmax denominators; normalize via
           reciprocal_approx_fast + partition_broadcast.
  stage 3 (per batch): AllToAll (512KB/rank, bf16) -> y^T [1024ch x
           256tok] slice, out = y^T.T @ W_proj (bf16) accumulated over 8
           channel chunks.
"""

import os
import numpy as np
import ml_dtypes

from concourse import bass, bacc, mybir, tile
from concourse.bass_utils import run_bass_kernel_spmd

F32 = mybir.dt.float32
F32R = mybir.dt.float32r
BF16 = mybir.dt.bfloat16

B, T, C = 4, 2048, 1024
H, D = 16, 64
NCORES = 8
HPC = H // NCORES            # heads per core = 2
QKC = HPC * D                # per-core q/k/v channels = 128
BT = B * T                   # 8192 tokens total
P = 128
TW = 512                     # token window for stage1/attention q windows
NW = T // TW                 # windows per batch = 4
NKT = T // P                 # kt tiles per batch = 16
SHT = T // NCORES            # tokens per shard per batch = 256
KC = C // P                  # 8 contraction chunks

PSS_BUFS = int(os.environ.get("KPSS", "2"))
PSY_BUFS = int(os.environ.get("KPSY", "2"))


def _causal_mask_01() -> np.ndarray:
    """mask[p, m, f] = 1.0 iff kt_local = 128*m + p <= f, for q windows of 512."""
    m = np.zeros((P, NW, TW), dtype=np.float32)
    p = np.arange(P)[:, None, None]
    mm = np.arange(NW)[None, :, None]
    f = np.arange(TW)[None, None, :]
    m[(P * mm + p) <= f] = 1.0
    return m.astype(ml_dtypes.bfloat16)


def build() -> bass.Bass:
    nc = bacc.Bacc(num_devices=NCORES, target_bir_lowering=False)

    x_d = nc.dram_tensor("x", [BT, C], BF16, kind="ExternalInput")
    wqkv_d = nc.dram_tensor("wqkv", [C, 3 * QKC], BF16, kind="ExternalInput")
    wproj_d = nc.dram_tensor("wproj", [C, C], BF16, kind="ExternalInput")
    out_d = nc.dram_tensor("out", [B * SHT, C], F32, kind="ExternalOutput")

    mask_d = nc.inline_tensor(_causal_mask_01(), name="mask01")
    ident_d = nc.inline_tensor(np.eye(P, dtype=np.float32).astype(ml_dtypes.bfloat16), name="ident")

    with tile.TileContext(nc) as tc:
        from contextlib import ExitStack

        with ExitStack() as ctx:
            # ---- persistent pools ----
            wq_pool = ctx.enter_context(tc.tile_pool(name="wq", bufs=1))
            msk_pool = ctx.enter_context(tc.tile_pool(name="msk", bufs=1))
            wp_pool = ctx.enter_context(tc.tile_pool(name="wp", bufs=1))
            dram = ctx.enter_context(tc.tile_pool(name="dram", bufs=1, space="DRAM"))

            ident_sb = msk_pool.tile([P, P], BF16)
            nc.sync.dma_start(out=ident_sb[:], in_=ident_d[:, :])
            wqkv_sb = wq_pool.tile([P, KC, 3 * QKC], BF16)
            nc.sync.dma_start(
                out=wqkv_sb[:], in_=wqkv_d[:, :].rearrange("(k p) n -> p k n", p=P)
            )
            mask_sb = msk_pool.tile([P, NW, TW], BF16)
            nc.sync.dma_start(out=mask_sb[:], in_=mask_d[:, :, :])
            wproj_sb = wp_pool.tile([P, KC, C], BF16)

            y_sends = [
                dram.tile([NCORES, QKC, SHT], BF16, tag=f"ys{b}", name=f"y_send{b}")
                for b in range(B)
            ]
            y_recvs = [
                dram.tile([NCORES, QKC, SHT], BF16, tag=f"yr{b}", name=f"y_recv{b}")
                for b in range(B)
            ]

            s12 = ExitStack()
            with s12:
                xT_pool = s12.enter_context(tc.tile_pool(name="xT", bufs=2))
                qkv_pool = s12.enter_context(tc.tile_pool(name="qkv", bufs=2))
                ps1 = s12.enter_context(
                    tc.tile_pool(name="ps1", bufs=2, space="PSUM")
                )
                pss = s12.enter_context(
                    tc.tile_pool(name="pss", bufs=PSS_BUFS, space="PSUM")
                )
                psy = s12.enter_context(
                    tc.tile_pool(name="psy", bufs=PSY_BUFS, space="PSUM")
                )
                pso = s12.enter_context(
                    tc.tile_pool(name="pso", bufs=1, space="PSUM")
                )
                pt_pool = s12.enter_context(tc.tile_pool(name="pt", bufs=6))
                nrm_pool = s12.enter_context(tc.tile_pool(name="nrm", bufs=2))
                yt_pool = s12.enter_context(tc.tile_pool(name="yt", bufs=2))
                yr_pool = s12.enter_context(tc.tile_pool(name="yr", bufs=2))
                ob_pool = s12.enter_context(tc.tile_pool(name="ob", bufs=2))

                for b in range(B):
                    qT_b = qkv_pool.tile([P, T], BF16, tag="qT")
                    kT_b = qkv_pool.tile([P, T], BF16, tag="kT")
                    v_b = qkv_pool.tile([P, NKT, HPC, D + 1], BF16, tag="v")
                    # ones column for softmax denominators
                    nc.gpsimd.memset(v_b[:, :, :, D : D + 1], 1.0)

                    # ---- stage 1: qT, kT, v for batch b ----
                    for w in range(NW):
                        t0 = b * T + w * TW
                        # x^T tiles arrive directly via the DMA transpose xbar
                        xT = xT_pool.tile([P, KC, TW], BF16)
                        for kc in range(KC):
                            nc.sync.dma_start_transpose(
                                out=xT[:, kc, :],
                                in_=x_d[t0 : t0 + TW, kc * P : (kc + 1) * P],
                            )
                        for which, dst in ((0, qT_b), (1, kT_b)):
                            ps = ps1.tile([P, TW], F32, tag="ps1")
                            for kc in range(KC):
                                nc.tensor.matmul(
                                    ps[:],
                                    lhsT=wqkv_sb[:, kc, which * QKC : (which + 1) * QKC],
                                    rhs=xT[:, kc, :],
                                    start=(kc == 0),
                                    stop=(kc == KC - 1),
                                )
                            nc.vector.tensor_copy(dst[:, w * TW : (w + 1) * TW], ps[:])
                        ps_vT = ps1.tile([P, TW], F32, tag="ps1", name="ps_vT")
                        for kc in range(KC):
                            nc.tensor.matmul(
                                ps_vT[:],
                                lhsT=wqkv_sb[:, kc, 2 * QKC : 3 * QKC],
                                rhs=xT[:, kc, :],
                                start=(kc == 0),
                                stop=(kc == KC - 1),
                            )
                        vT_sb = xT_pool.tile([P, TW], BF16, tag="vT", name="vT_sb")
                        nc.vector.tensor_copy(vT_sb[:], ps_vT[:])
                        ps_v = ps1.tile([P, TW], BF16, tag="psv", name="ps_v", bufs=1)
                        for s in range(TW // P):
                            nc.tensor.transpose(
                                ps_v[:, s * P : (s + 1) * P],
                                vT_sb[:, s * P : (s + 1) * P],
                                ident_sb[:],
                            )
                        jt0 = w * (TW // P)
                        nc.vector.tensor_copy(
                            v_b[:, jt0 : jt0 + TW // P, :, 0:D],
                            ps_v[:].rearrange("p (s h d) -> p s h d", s=TW // P, h=HPC),
                        )

                    if b == 0:
                        # deferred so startup DMA bandwidth goes to x / wqkv
                        nc.sync.dma_start(
                            out=wproj_sb[:],
                            in_=wproj_d[:, :].rearrange("(k p) n -> p k n", p=P),
                        )

                    # ---- stage 2: attention for batch b ----
                    # h innermost: two independent AV chains overlap on PE.
                    # Denominators are staged at partition offsets {0,32,64,96}
                    # of a shared tile so ONE DVE reciprocal (a ~6-pass
                    # microcoded op whose cost scales with free size, not
                    # partitions) serves 4 chains; drains run per group of 4.
                    def emit_stage3(bb):
                        yr = yr_pool.tile([P, KC, SHT], BF16, tag="yr")
                        nc.sync.dma_start(
                            out=yr[:],
                            in_=y_recvs[bb][:, :, :].rearrange("k p t -> p k t"),
                        )
                        for jt in range(SHT // P):
                            for half in range(C // TW):
                                ps_o = pso.tile([P, TW], F32, tag="ps_o")
                                for kc in range(KC):
                                    nc.tensor.matmul(
                                        ps_o[:],
                                        lhsT=yr[:, kc, jt * P : (jt + 1) * P],
                                        rhs=wproj_sb[
                                            :, kc, half * TW : (half + 1) * TW
                                        ],
                                        start=(kc == 0),
                                        stop=(kc == KC - 1),
                                    )
                                ob = ob_pool.tile([P, TW], F32, tag="ob")
                                nc.vector.tensor_copy(ob[:], ps_o[:])
                                nc.sync.dma_start(
                                    out=out_d[
                                        bb * SHT + jt * P : bb * SHT + (jt + 1) * P,
                                        half * TW : (half + 1) * TW,
                                    ],
                                    in_=ob[:],
                                )

                    yus = [None] * (NW * HPC)
                    den4 = [None, None]
                    rec4 = [None, None]
                    for w in range(NW):
                        # the previous batch's projection slots in here so its
                        # AllToAll latency hides under this batch's attention
                        # (the Tensor queue is in-order; emitting stage3(b-1)
                        # after stage2(b) would stall stage1(b+1) behind it)
                        if w == 1 and b >= 1:
                            emit_stage3(b - 1)
                        for h in range(HPC):
                            j = w * HPC + h
                            g, slot = j // 4, j % 4
                            if slot == 0:
                                den4[g] = nrm_pool.tile(
                                    [3 * 32 + 1, TW], F32, tag=f"den4_{g}", bufs=1, name=f"den4_{g}"
                                )
                            qT_h = qT_b[h * D : (h + 1) * D, :]
                            kT_h = kT_b[h * D : (h + 1) * D, :]
                            nkt = (w + 1) * (TW // P)
                            ps_y = psy.tile([D + 1, TW], F32, tag="ps_y")
                            # diagonal (masked) tiles first so their longer
                            # exp+mask path pipelines under later S matmuls
                            jks = list(range(w * (TW // P), nkt)) + list(
                                range(0, w * (TW // P))
                            )
                            for ji, jk in enumerate(jks):
                                ps_s = pss.tile([P, TW], F32, tag="ps_s")
                                nc.tensor.matmul(
                                    ps_s[:],
                                    lhsT=kT_h[:, jk * P : (jk + 1) * P],
                                    rhs=qT_h[:, w * TW : (w + 1) * TW],
                                    start=True,
                                    stop=True,
                                )
                                pt = pt_pool.tile([P, TW], BF16, tag="pt")
                                nc.scalar.activation(
                                    pt[:],
                                    ps_s[:],
                                    mybir.ActivationFunctionType.Exp,
                                    scale=1.0 / np.sqrt(D),
                                )
                                m = jk - w * (TW // P)
                                if m >= 0:
                                    nc.vector.tensor_mul(
                                        pt[:], pt[:], mask_sb[:, m, :]
                                    )
                                nc.tensor.matmul(
                                    ps_y[:],
                                    lhsT=v_b[:, jk, h, :],
                                    rhs=pt[:],
                                    start=(ji == 0),
                                    stop=(ji == nkt - 1),
                                )
                            yu = yt_pool.tile([D + 1, TW], F32, tag=f"yu{j}", bufs=1)
                            nc.vector.tensor_copy(yu[:], ps_y[:])
                            nc.vector.tensor_copy(
                                den4[g][32 * slot : 32 * slot + 1, :],
                                yu[D : D + 1, :],
                            )
                            yus[j] = yu
                            if slot == 3:
                                rec4[g] = nrm_pool.tile(
                                    [3 * 32 + 1, TW], F32, tag=f"rec4_{g}", bufs=1, name=f"rec4_{g}"
                                )
                                nc.vector.reciprocal(rec4[g][:], den4[g][:])
                                for jd in range(g * 4, g * 4 + 4):
                                    wd, hd = jd // HPC, jd % HPC
                                    sd = jd % 4
                                    # partition_broadcast only reads partition
                                    # 0, so bounce the recip row down first
                                    r1 = nrm_pool.tile([1, TW], F32, tag="r1")
                                    nc.vector.tensor_copy(
                                        r1[:], rec4[g][32 * sd : 32 * sd + 1, :]
                                    )
                                    bc = nrm_pool.tile([D, TW], F32, tag="bc")
                                    nc.gpsimd.partition_broadcast(bc[:], r1[:])
                                    yt = yt_pool.tile([D, TW], BF16, tag="yt")
                                    nc.vector.tensor_mul(
                                        yt[:], yus[jd][0:D, :], bc[:]
                                    )
                                    for half in range(2):
                                        nc.sync.dma_start(
                                            out=y_sends[b][
                                                2 * wd + half,
                                                hd * D : (hd + 1) * D,
                                                :,
                                            ],
                                            in_=yt[:, half * SHT : (half + 1) * SHT],
                                        )

                    # ---- A2A for batch b; its projection is emitted inside
                    # batch b+1's stage 2 (or right here for the last batch)
                    nc.gpsimd.collective_compute(
                        "AllToAll",
                        mybir.AluOpType.bypass,
                        replica_groups=[list(range(NCORES))],
                        ins=[y_sends[b].opt()],
                        outs=[y_recvs[b].opt()],
                    )
                    if b == B - 1:
                        emit_stage3(b)

    nc.finalize()
    return nc


_NC_CACHE: dict = {}


def _get_nc() -> bass.Bass:
    if "nc" not in _NC_CACHE:
        _NC_CACHE["nc"] = build()
    return _NC_CACHE["nc"]


def shard_inputs(x, W_qkv, W_proj):
    x = np.ascontiguousarray(
        np.asarray(x, dtype=np.float32).reshape(BT, C).astype(ml_dtypes.bfloat16)
    )
    W_qkv = np.asarray(W_qkv, dtype=np.float32)
    W_proj = np.ascontiguousarray(
        np.asarray(W_proj, dtype=np.float32).astype(ml_dtypes.bfloat16)
    )
    in_maps = []
    for c in range(NCORES):
        cols = slice(QKC * c, QKC * (c + 1))
        w_c = np.ascontiguousarray(
            np.concatenate(
                [W_qkv[:, cols], W_qkv[:, C:][:, cols], W_qkv[:, 2 * C :][:, cols]],
                axis=1,
            ).astype(ml_dtypes.bfloat16)
        )
        in_maps.append({"x": x, "wqkv": w_c, "wproj": W_proj})
    return in_maps


def run(in_maps, trace=False, **kwargs):
    return run_bass_kernel_spmd(
        _get_nc(), in_maps, core_ids=list(range(NCORES)), trace=trace, **kwargs
    )


def unshard(res) -> np.ndarray:
    """Core c owns tokens [c*SHT, (c+1)*SHT) of every batch."""
    out = np.empty((B, T, C), dtype=np.float32)
    for c in range(NCORES):
        oc = np.asarray(res.results[c]["out"]).reshape(B, SHT, C)
        out[:, c * SHT : (c + 1) * SHT, :] = oc
    return out


def kernel(x, W_qkv, W_proj):
    res = run(shard_inputs(x, W_qkv, W_proj), trace=False)
    return unshard(res)
